# revision 1
# baseline (speedup 1.0000x reference)
"""DGMNet forward pass on 8 Trainium2 NeuronCores.

Data-parallel: the 131072-point batch is split into 8 shards of 16384; all
weights are replicated (about 12 MB, resident in SBUF for the whole kernel).

Per-core layout is feature-major ("transposed"): every activation tensor is
kept as 4 chunks of [128 nodes (partitions) x B_TILE samples (free dim)], so
the matmuls chain without any on-chip transposes:

    H^T = W^T @ X^T   -> out = lhsT.T @ rhs with lhsT = W[k, m], rhs = X^T

Per gate-chunk, the X-contribution (K=4) and the four H-contribution
K-chunks (K=128 each) accumulate into a single PSUM bank; ScalarE applies
the activation with the per-partition bias straight out of PSUM. silu(x)
is computed as x*sigmoid(x) (ACT sigmoid + one fused VectorE
scalar_tensor_tensor (ps+b)*s) so ACT only ever evaluates Sigmoid/Tanh —
both live in one activation table, avoiding the ~1.3us table reload a Silu
would force at every function switch.

Two batch-tiles are processed interleaved phase-by-phase so one tile's
ACT/DVE tail at each layer boundary hides behind the other tile's matmuls.

Matmuls default to float32r (reduced-precision fp32). fp32r operands must
be produced *rounded*: weights get a one-time DVE copy into fp32r tiles;
activation tiles that feed matmuls are written with fp32r output dtype by
their producing op.
"""

import numpy as np

import concourse.bass as bass
import concourse.mybir as mybir
import concourse.tile as tile
from concourse import bacc
from concourse.bass import ds, ts
from concourse.bass_utils import run_bass_kernel_spmd

N_CORES = 8
BATCH = 131072
B_CORE = BATCH // N_CORES  # 16384
B_TILE = 512
N_TILES = B_CORE // B_TILE  # 32
L = 3  # layers
NN = 512  # nodes
D = 4  # input dim
NCH = NN // 128  # node chunks of 128

F32 = mybir.dt.float32
F32R = mybir.dt.float32r
BF16 = mybir.dt.bfloat16
AF = mybir.ActivationFunctionType
ALU = mybir.AluOpType


def build_kernel(n_passes: int = 1, mm_dtype: str = "f32r",
                 no_mm: bool = False, n_tiles: int = N_TILES,
                 loop_mode: str = "static", staggered: bool = False):
    """Build the per-core Bass program. n_passes > 1 repeats the whole
    compute (for timing); output is identical. no_mm is a timing-ablation
    knob that produces WRONG output.
    X is expected HOST-TRANSPOSED as [D, B_CORE] (contiguous loads)."""
    MMDT = {"f32r": F32R, "bf16": BF16}[mm_dtype]
    nc = bacc.Bacc("TRN2", target_bir_lowering=False, debug=False, num_devices=N_CORES)

    X = nc.dram_tensor("X", [D, B_CORE], F32, kind="ExternalInput")
    W_in = nc.dram_tensor("W_in", [D, NN], F32, kind="ExternalInput")
    b_in = nc.dram_tensor("b_in", [1, NN], F32, kind="ExternalInput")
    Wf = nc.dram_tensor("Wf", [L, D, NN], F32, kind="ExternalInput")
    Uf = nc.dram_tensor("Uf", [L, NN, NN], F32, kind="ExternalInput")
    bf = nc.dram_tensor("bf", [L, 1, NN], F32, kind="ExternalInput")
    Wu = nc.dram_tensor("Wu", [L, D, NN], F32, kind="ExternalInput")
    Uu = nc.dram_tensor("Uu", [L, NN, NN], F32, kind="ExternalInput")
    bu = nc.dram_tensor("bu", [L, 1, NN], F32, kind="ExternalInput")
    Wo1 = nc.dram_tensor("Wo1", [L, D, NN], F32, kind="ExternalInput")
    Uo1 = nc.dram_tensor("Uo1", [L, NN, NN], F32, kind="ExternalInput")
    bo1 = nc.dram_tensor("bo1", [L, 1, NN], F32, kind="ExternalInput")
    Wo2 = nc.dram_tensor("Wo2", [L, NN, NN], F32, kind="ExternalInput")
    bo2 = nc.dram_tensor("bo2", [L, 1, NN], F32, kind="ExternalInput")
    W_out = nc.dram_tensor("W_out", [NN, 1], F32, kind="ExternalInput")
    b_out = nc.dram_tensor("b_out", [1, 1], F32, kind="ExternalInput")
    out = nc.dram_tensor("out", [1, B_CORE], F32, kind="ExternalOutput")

    from contextlib import ExitStack
    with tile.TileContext(nc) as tc, ExitStack() as stack:
        wpool = stack.enter_context(tc.tile_pool(name="weights", bufs=1))
        wstack = ExitStack()
        stage = wstack.enter_context(tc.tile_pool(name="stage", bufs=2))

        def rounded(t, s):
            nc.vector.tensor_copy(t[:], s[:])
            return t

        def load_w(src, tag):
            """[D, NN] input-side weight, rounded to MMDT."""
            s = stage.tile([D, NN], F32, tag="stage_s", name="stage_s")
            nc.sync.dma_start(s[:], src)
            return rounded(wpool.tile([D, NN], MMDT, tag=tag, name=tag), s)

        def load_u(src, tag):
            """U-type [512, 512] -> [128, (ko n)]: lhsT chunk (ko, m) is
            [:, ko*NN + m*128 ...]."""
            t = wpool.tile([128, NCH * NN], MMDT, tag=tag, name=tag)
            s = stage.tile([128, NCH * NN], F32, tag="stage_u", name="stage_u")
            nc.sync.dma_start(s[:].rearrange("p (ko n) -> p ko n", ko=NCH),
                              src.rearrange("(ko p) n -> p ko n", p=128))
            return rounded(t, s)

        def load_b(src, tag):
            """bias [1, NN] -> per-partition columns [128, NCH]."""
            t = wpool.tile([128, NCH], F32, tag=tag, name=tag)
            nc.sync.dma_start(t[:], src.rearrange("one (m p) -> one p m", p=128)[0])
            return t

        win_sb = load_w(W_in[:, :], "win")
        bin_sb = load_b(b_in[:, :], "bin")
        wf_sb, wu_sb, wo1_sb = [], [], []
        uf_sb, uu_sb, uo1_sb, wo2_sb = [], [], [], []
        bf_sb, bu_sb, bo1_sb, bo2_sb = [], [], [], []
        for i in range(L):
            wf_sb.append(load_w(Wf[i], f"wf{i}"))
            wu_sb.append(load_w(Wu[i], f"wu{i}"))
            wo1_sb.append(load_w(Wo1[i], f"wo1{i}"))
            uf_sb.append(load_u(Uf[i], f"uf{i}"))
            uu_sb.append(load_u(Uu[i], f"uu{i}"))
            uo1_sb.append(load_u(Uo1[i], f"uo1{i}"))
            wo2_sb.append(load_u(Wo2[i], f"wo2{i}"))
            bf_sb.append(load_b(bf[i], f"bf{i}"))
            bu_sb.append(load_b(bu[i], f"bu{i}"))
            bo1_sb.append(load_b(bo1[i], f"bo1{i}"))
            bo2_sb.append(load_b(bo2[i], f"bo2{i}"))
        wout_sb = wpool.tile([128, NCH], MMDT, tag="wout", name="wout")
        ws = stage.tile([128, NCH], F32, tag="stage_w", name="stage_w")
        nc.sync.dma_start(ws[:].rearrange("p (k o) -> p k o", o=1),
                          W_out.rearrange("(ko p) one -> p ko one", p=128))
        rounded(wout_sb, ws)
        bout_sb = wpool.tile([1, 1], F32, tag="bout", name="bout")
        nc.sync.dma_start(bout_sb[:], b_out[:, :])
        wstack.close()  # release the staging pool's SBUF before steady-state pools

        xt_pool = stack.enter_context(tc.tile_pool(name="xt", bufs=1))
        h_pool = stack.enter_context(tc.tile_pool(name="h", bufs=1))
        g_pool = stack.enter_context(tc.tile_pool(name="gates", bufs=1))
        ps_pool = stack.enter_context(tc.tile_pool(name="psum", bufs=6, space="PSUM"))
        po_pool = stack.enter_context(tc.tile_pool(name="psum_out", bufs=2, space="PSUM"))
        o_pool = stack.enter_context(tc.tile_pool(name="out", bufs=2))

        gates = (
            (wu_sb, uu_sb, bu_sb, AF.Sigmoid, "u", MMDT),
            (wo1_sb, uo1_sb, bo1_sb, AF.Tanh, "o1", MMDT),
            (wf_sb, uf_sb, bf_sb, AF.Sigmoid, "f", F32),
        )

        def mm(psum, lhsT, rhs, start, stop):
            if no_mm:
                if start:
                    nc.vector.memset(psum, 0.0)
                return
            nc.tensor.matmul(psum, lhsT, rhs, start=start, stop=stop)

        def silu_from(ps, bias, out_tile, s_tile):
            """out = silu(ps + bias): ACT sigmoid + fused (ps+b)*s on DVE."""
            nc.scalar.activation(s_tile[:], ps[:], AF.Sigmoid, bias=bias)
            nc.vector.scalar_tensor_tensor(
                out_tile[:], ps[:], bias, s_tile[:],
                op0=ALU.add, op1=ALU.mult)

        def load_x(it, slot):
            def tg(base):
                return f"{base}s{slot}"
            boff = it * B_TILE
            xt_f = xt_pool.tile([D, B_TILE], F32, tag=tg("xtf"), name="xt_f")
            nc.sync.dma_start(xt_f[:], X[:, ds(boff, B_TILE)])
            xt = xt_pool.tile([D, B_TILE], MMDT, tag=tg("xt"), name="xt")
            nc.vector.tensor_copy(xt[:], xt_f[:])
            return xt

        def h_init(xt, slot):
            def tg(base):
                return f"{base}s{slot}"
            h = []
            for m in range(NCH):
                ps = ps_pool.tile([128, B_TILE], F32, tag="ps", name="ps_h")
                mm(ps[:], win_sb[:, ts(m, 128)], xt[:], start=True, stop=True)
                s = g_pool.tile([128, B_TILE], F32, tag=tg(f"o1{m}"), name="s_h")
                t = h_pool.tile([128, B_TILE], MMDT, tag=tg(f"h{m}"), name="h0")
                silu_from(ps, bin_sb[:, m:m + 1], t, s)
                h.append(t)
            return h

        def layer(i, xt, h, slot):
            def tg(base):
                return f"{base}s{slot}"
            gt = {}
            # gate order u, o1, f: g = u*o1 is computed while f's matmuls
            # stream, so the o2 accumulation never waits on the DVE
            for w_sb, u_sb, b_sb, fn, nm, dt_g in gates:
                for m in range(NCH):
                    ps = ps_pool.tile([128, B_TILE], F32, tag="ps", name="ps_g")
                    mm(ps[:], w_sb[i][:, ts(m, 128)], xt[:], start=True, stop=False)
                    for k in range(NCH):
                        mm(ps[:], u_sb[i][:, k * NN + m * 128: k * NN + (m + 1) * 128],
                           h[k][:], start=False, stop=(k == NCH - 1))
                    t = g_pool.tile([128, B_TILE], dt_g, tag=tg(f"{nm}{m}"), name="gate")
                    nc.scalar.activation(t[:], ps[:], fn, bias=b_sb[i][:, m:m + 1])
                    gt[nm, m] = t
                    if nm == "o1":
                        # g = u * o1 in place over u's tile (u dead after)
                        gm = gt["u", m]
                        nc.vector.tensor_mul(gm[:], gm[:], t[:])
            g = [gt["u", m] for m in range(NCH)]
            hn = []
            for m in range(NCH):
                ps = ps_pool.tile([128, B_TILE], F32, tag="ps", name="ps_o2")
                for k in range(NCH):
                    mm(ps[:], wo2_sb[i][:, k * NN + m * 128: k * NN + (m + 1) * 128],
                       g[k][:], start=(k == 0), stop=(k == NCH - 1))
                s = g_pool.tile([128, B_TILE], F32, tag=tg(f"o1{m}"), name="s_o2")
                o2 = s  # (ps+b)*s written in place over the sigmoid tile
                silu_from(ps, bo2_sb[i][:, m:m + 1], o2, s)
                # f*h in place over f's tile (f dead after this)
                fh = gt["f", m]
                nc.vector.tensor_mul(fh[:], fh[:], h[m][:])
                t = h_pool.tile([128, B_TILE], MMDT, tag=tg(f"h{m}"), name="hn")
                nc.vector.tensor_add(t[:], fh[:], o2[:])
                hn.append(t)
            return hn

        def out_stage(it, h, slot):
            def tg(base):
                return f"{base}s{slot}"
            boff = it * B_TILE
            po = po_pool.tile([1, B_TILE], F32, tag="po", name="po")
            for k in range(NCH):
                mm(po[:], wout_sb[:, k:k + 1], h[k][:], start=(k == 0), stop=(k == NCH - 1))
            so = o_pool.tile([1, B_TILE], F32, tag=tg("so"), name="so")
            silu_from(po, bout_sb[0:1, 0:1], so, so)
            nc.sync.dma_start(out[:, ds(boff, B_TILE)], so[:])

        def body_pair(it_a, it_b):
            # two tiles interleaved phase-by-phase: each tile's ACT/DVE tail
            # at a phase boundary is hidden behind the other tile's matmuls
            xa = load_x(it_a, 0)
            xb = load_x(it_b, 1)
            ha = h_init(xa, 0)
            hb = h_init(xb, 1)
            for i in range(L):
                ha = layer(i, xa, ha, 0)
                hb = layer(i, xb, hb, 1)
            out_stage(it_a, ha, 0)
            out_stage(it_b, hb, 1)

        for _ in range(n_passes):
            if loop_mode == "static":
                for it in range(0, n_tiles, 2):
                    body_pair(it, it + 1)
            else:
                with tc.For_i(0, n_tiles // 2, 1, hint_engines=(mybir.EngineType.PE,),
                              staggered_reset=staggered) as it:
                    body_pair(it * 2, it * 2 + 1)

    nc.compile()
    return nc


_NC_CACHE = {}


def _get_nc(n_passes=1, mm_dtype="f32r", **kw):
    key = (n_passes, mm_dtype, tuple(sorted(kw.items())))
    if key not in _NC_CACHE:
        _NC_CACHE[key] = build_kernel(n_passes, mm_dtype, **kw)
    return _NC_CACHE[key]


def make_in_maps(inputs: dict):
    """Shard X (host-transposed per core) and replicate weights."""
    X = np.asarray(inputs["X"], dtype=np.float32)
    shared = {
        k: np.ascontiguousarray(np.asarray(v, dtype=np.float32))
        for k, v in inputs.items() if k != "X"
    }
    return [
        {"X": np.ascontiguousarray(X[c * B_CORE:(c + 1) * B_CORE].T), **shared}
        for c in range(N_CORES)
    ]


def run(inputs: dict, n_passes: int = 1, mm_dtype: str = "f32r", **kw):
    """Shard, run on 8 cores, gather. Returns (full_output, results_obj)."""
    nc = _get_nc(n_passes, mm_dtype, **kw)
    in_maps = make_in_maps(inputs)
    res = run_bass_kernel_spmd(nc, in_maps, core_ids=list(range(N_CORES)))
    full = np.concatenate(
        [res.results[c]["out"].reshape(B_CORE, 1) for c in range(N_CORES)], axis=0)
    return full, res


def kernel(**inputs) -> np.ndarray:
    full, _ = run(inputs)
    return full

